# revision 1
# baseline (speedup 1.0000x reference)
"""DeformConv2d (DCNv2) on 8 Trainium2 NeuronCores.

Data-parallel over batch: one image per core.

Per-core pipeline:
  1. Build channel-last bf16 table2 in DRAM: row p = [img[:, p], img[:, p+64]]
     (channels at spatial p and at the row below). One 2KB dma_gather
     descriptor at row idx then covers all 4 bilinear neighbors
     (x-pair contiguous via elem_step=512, y-pair packed in the row).
  2. dma_gather 9*4096 sample rows -> tiles [s%128 partitions, 4*256 ch].
  3. Blend on DVE: 4 fused tensor_scalar / scalar_tensor_tensor ops with
     per-partition (= per output position) bilinear*valid*mask weights.
  4. PE transpose blended cols [s,c] -> [c,s], GEMM (contraction 2304 = 9k x
     2 c-halves x 128) accumulating in PSUM, bias add, fp32 out.

Host precomputes gather indices + folded bilinear weights from offset/mask
(small: 18*64*64 per image) and packs weight/bias into GEMM layout.
"""
import os
import sys
from contextlib import ExitStack

import numpy as np
import ml_dtypes

import concourse.bass as bass
import concourse.mybir as mybir
from concourse import bacc
from concourse.ap import AP
from concourse.tile import TileContext
from concourse.bass_utils import run_bass_kernel_spmd

N_CORES = 8
N, CIN, HH, WW = 8, 256, 64, 64
COUT = 256
KK = 9
S = HH * WW                 # 4096 output positions
TROWS = 4160                # table2 rows (>= 4097), 512 bf16 each
ROW = 512                   # elements per table2 row
ELEM = 1024                 # elements per gather descriptor (2 rows)
NQ = 4                      # s-quarters
SQ = S // NQ                # 1024 positions per quarter
CHUNK = SQ                  # idxs per dma_gather call
JL = SQ // 128              # 8 j-chunks per quarter
NIDX = KK * S               # 36864 gathers per image
NCHUNK = 18                 # contraction chunks (9 k * 2 c-halves)

bf16 = mybir.dt.bfloat16
f32 = mybir.dt.float32
i16 = mybir.dt.int16
MUL = mybir.AluOpType.mult
ADD = mybir.AluOpType.add

_PROGRAM = None


def _build_program():
    nc = bacc.Bacc("TRN2", target_bir_lowering=False, debug=False,
                   num_devices=N_CORES)

    x = nc.dram_tensor("x", [CIN, S], f32, kind="ExternalInput")
    idx_d = nc.dram_tensor("idx", [128, NIDX // 16], i16, kind="ExternalInput")
    wq_d = nc.dram_tensor("wq", [128, NQ * KK * 4 * JL], f32, kind="ExternalInput")
    wg_d = nc.dram_tensor("wg", [128, NCHUNK * 256], bf16, kind="ExternalInput")
    bias_d = nc.dram_tensor("bias", [128, 2], f32, kind="ExternalInput")
    y = nc.dram_tensor("y", [COUT, S], f32, kind="ExternalOutput")
    table2 = nc.dram_tensor("table2", [TROWS, ROW], bf16)

    # overlapping gather source AP: row i covers elements [i*512, i*512+1024)
    src_ap = AP(tensor=table2, offset=0, ap=[[ROW, TROWS - 2], [1, ELEM]])

    with TileContext(nc) as tc, ExitStack() as stk:
        const_pool = stk.enter_context(tc.tile_pool(name="const", bufs=1))
        big_pool = stk.enter_context(tc.tile_pool(name="big", bufs=3))
        imgb_pool = stk.enter_context(tc.tile_pool(name="imgb", bufs=2))
        tt_pool = stk.enter_context(tc.tile_pool(name="tt", bufs=3))
        acc_pool = stk.enter_context(tc.tile_pool(name="acc", bufs=6))
        cols_pool = stk.enter_context(tc.tile_pool(name="cols", bufs=2))
        out_pool = stk.enter_context(tc.tile_pool(name="out", bufs=3))
        ps_pool = stk.enter_context(tc.tile_pool(name="ps", bufs=3, space="PSUM"))
        pg_pool = stk.enter_context(tc.tile_pool(name="pg", bufs=4, space="PSUM"))

        # ---- constants / parameters -------------------------------------
        idx_sb = const_pool.tile([128, NIDX // 16], i16)
        nc.sync.dma_start(out=idx_sb[:], in_=idx_d[:])
        wq_sb = const_pool.tile([128, NQ * KK * 4 * JL], f32)
        nc.sync.dma_start(out=wq_sb[:], in_=wq_d[:])
        wg_sb = const_pool.tile([128, NCHUNK * 256], bf16)
        nc.sync.dma_start(out=wg_sb[:], in_=wg_d[:])
        bias_sb = const_pool.tile([128, 2], f32)
        nc.sync.dma_start(out=bias_sb[:], in_=bias_d[:])

        ident = const_pool.tile([128, 128], bf16)
        nc.vector.memset(ident[:], 1.0)
        nc.gpsimd.affine_select(
            ident[:], ident[:], pattern=[[1, 128]],
            compare_op=mybir.AluOpType.is_equal, fill=0.0,
            base=0, channel_multiplier=-1,
        )

        # ---- zero the table tail (rows 4032+: y+1 slots / pad row) ------
        ztile = const_pool.tile([128, ROW], bf16)
        nc.vector.memset(ztile[:], 0.0)
        nc.sync.dma_start(out=table2[4032:4160, :], in_=ztile[:])

        # ---- build table2: transpose image to channel-last bf16 ---------
        # table2[p, 0:256]   = img[:, p]
        # table2[p, 256:512] = img[:, p + 64]   (rows < 4032)
        for cc in range(2):
            in_sb = big_pool.tile([128, S], f32, tag="big")
            nc.sync.dma_start(out=in_sb[:], in_=x[cc * 128:(cc + 1) * 128, :])
            imgb = imgb_pool.tile([128, S], bf16)
            nc.vector.tensor_copy(imgb[:], in_sb[:])
            for g in range(4):
                tt = tt_pool.tile([128, 8, 128], bf16)
                for t in range(8):
                    ps = ps_pool.tile([128, 4, 128], bf16)
                    nc.tensor.transpose(
                        ps[:, 0, :], imgb[:, (g * 8 + t) * 128:(g * 8 + t + 1) * 128],
                        ident[:])
                    nc.scalar.copy(tt[:, t, :], ps[:, 0, :])
                hw0 = g * 1024
                # first half: rows hw0 + t*128 + p, cols cc*128..+128
                dst1 = AP(tensor=table2, offset=hw0 * ROW + cc * 128,
                          ap=[[ROW, 128], [ROW * 128, 8], [1, 128]])
                nc.sync.dma_start(out=dst1, in_=tt[:])
                # second half: rows hw0 - 64 + t*128 + p, cols 256+cc*128
                if g == 0:
                    dst2a = AP(tensor=table2, offset=256 + cc * 128,
                               ap=[[ROW, 64], [1, 128]])
                    nc.sync.dma_start(out=dst2a, in_=tt[64:128, 0, :])
                    dst2b = AP(tensor=table2, offset=64 * ROW + 256 + cc * 128,
                               ap=[[ROW, 128], [ROW * 128, 7], [1, 128]])
                    nc.sync.dma_start(out=dst2b, in_=tt[:, 1:8, :])
                else:
                    dst2 = AP(tensor=table2,
                              offset=(hw0 - 64) * ROW + 256 + cc * 128,
                              ap=[[ROW, 128], [ROW * 128, 8], [1, 128]])
                    nc.sync.dma_start(out=dst2, in_=tt[:])

        # ---- main loop ---------------------------------------------------
        for Q in range(NQ):
            cols_cs = cols_pool.tile([128, NCHUNK, SQ], bf16, tag="cols")
            for k in range(KK):
                cb = Q * KK + k
                g_t = big_pool.tile([128, JL, ELEM], bf16, tag="big")
                nc.gpsimd.dma_gather(
                    g_t[:], src_ap,
                    idx_sb[:, cb * (CHUNK // 16):(cb + 1) * (CHUNK // 16)],
                    CHUNK, CHUNK, ELEM, elem_step=ROW,
                )
                for jl in range(JL):
                    wcol = ((Q * KK + k) * 4) * JL + jl
                    acc = acc_pool.tile([128, 256], bf16, tag="acc")
                    nc.vector.tensor_scalar(
                        acc[:], g_t[:, jl, 0:256],
                        wq_sb[:, wcol:wcol + 1], None, MUL)
                    for q in (1, 2, 3):
                        wc = wcol + q * JL
                        nc.vector.scalar_tensor_tensor(
                            acc[:], g_t[:, jl, q * 256:(q + 1) * 256],
                            wq_sb[:, wc:wc + 1], acc[:], MUL, ADD)
                    # transpose [s,c] -> [c,s] per c-half
                    for ch in range(2):
                        ps = ps_pool.tile([128, 4, 128], bf16)
                        nc.tensor.transpose(
                            ps[:, jl % 4, :], acc[:, ch * 128:(ch + 1) * 128],
                            ident[:])
                        nc.vector.tensor_copy(
                            cols_cs[:, k * 2 + ch, jl * 128:(jl + 1) * 128],
                            ps[:, jl % 4, :])
            for b in range(2):
                for oc in range(2):
                    po = pg_pool.tile([128, 512], f32)
                    for chunk in range(NCHUNK):
                        nc.tensor.matmul(
                            po[:],
                            wg_sb[:, chunk * 256 + oc * 128:chunk * 256 + (oc + 1) * 128],
                            cols_cs[:, chunk, b * 512:(b + 1) * 512],
                            start=(chunk == 0), stop=(chunk == NCHUNK - 1))
                    osb = out_pool.tile([128, 512], f32)
                    nc.scalar.activation(
                        osb[:], po[:], mybir.ActivationFunctionType.Identity,
                        bias=bias_sb[:, oc:oc + 1])
                    nc.sync.dma_start(
                        out=y[oc * 128:(oc + 1) * 128,
                              Q * SQ + b * 512:Q * SQ + (b + 1) * 512],
                        in_=osb[:])

    nc.compile()
    return nc


def _host_prep(offset, mask):
    """Per-image gather indices + folded bilinear weights.

    Returns idx_sb [128, NIDX//16] i16 and wq_sb [128, NQ*KK*4*JL] f32.
    """
    off = offset.reshape(KK, 2, S).astype(np.float32)
    dy = off[:, 0]
    dx = off[:, 1]
    ky = (np.arange(KK, dtype=np.float32) // 3)[:, None]
    kx = (np.arange(KK, dtype=np.float32) % 3)[:, None]
    ho = np.float32(np.arange(S) // WW)[None, :]
    wo = np.float32(np.arange(S) % WW)[None, :]
    yy = ((ky + (ho - np.float32(1.0))).astype(np.float32) + dy).astype(np.float32)
    xx = ((kx + (wo - np.float32(1.0))).astype(np.float32) + dx).astype(np.float32)
    y0f = np.floor(yy)
    x0f = np.floor(xx)
    wy1 = yy - y0f
    wx1 = xx - x0f
    wy0 = np.float32(1.0) - wy1
    wx0 = np.float32(1.0) - wx1
    y0 = y0f.astype(np.int64)
    x0 = x0f.astype(np.int64)
    vy0 = (y0 >= 0) & (y0 < HH)
    vy1 = (y0 + 1 >= 0) & (y0 + 1 < HH)
    vx0 = (x0 >= 0) & (x0 < WW)
    vx1 = (x0 + 1 >= 0) & (x0 + 1 < WW)
    m = mask.reshape(KK, S).astype(np.float32)
    W00 = wy0 * wx0 * vy0 * vx0 * m
    W10 = wy1 * wx0 * vy1 * vx0 * m
    W01 = wy0 * wx1 * vy0 * vx1 * m
    W11 = wy1 * wx1 * vy1 * vx1 * m
    sy = y0 == -1
    sx = x0 == -1
    A00 = np.where(sy, np.where(sx, W11, W10), np.where(sx, W01, W00))
    A10 = np.where(sy, 0.0, np.where(sx, W11, W10))
    A01 = np.where(sx, 0.0, np.where(sy, W11, W01))
    A11 = np.where(sy | sx, 0.0, W11)
    y0c = np.clip(y0 + sy, 0, HH - 1)
    x0c = np.clip(x0 + sx, 0, WW - 1)
    idx = (y0c * WW + x0c).astype(np.int16)          # [KK, S]
    Wq = np.stack([A00, A10, A01, A11], 1).astype(np.float32)  # [KK, 4, S]

    # pack idx: call cb = Q*KK + k covers positions i (s = Q*SQ+i);
    # i -> partition i%16, col cb*64 + i//16; replicate x8 across partitions.
    idx_q = idx.reshape(KK, NQ, SQ)                   # [k, Q, i]
    idx_q = idx_q.transpose(1, 0, 2)                  # [Q, k, i]
    cols = idx_q.reshape(NQ * KK, SQ // 16, 16)       # [cb, col, row]
    idx16 = cols.transpose(2, 0, 1).reshape(16, NQ * KK * (SQ // 16))
    idx_sb = np.tile(idx16, (8, 1))

    # pack weights: col = ((Q*KK + k)*4 + q)*JL + jl; partition p = s%128
    w = Wq.reshape(KK, 4, NQ, JL, 128)                # [k, q, Q, jl, p]
    w = w.transpose(4, 2, 0, 1, 3)                    # [p, Q, k, q, jl]
    wq_sb = np.ascontiguousarray(w.reshape(128, NQ * KK * 4 * JL))
    return idx_sb, wq_sb


def kernel(input, offset, mask, weight, bias):
    global _PROGRAM
    if _PROGRAM is None:
        _PROGRAM = _build_program()
    nc = _PROGRAM

    wg = weight.reshape(COUT, CIN, KK).astype(np.float32)
    wg = wg.transpose(1, 2, 0).reshape(2, 128, KK, COUT)
    wg = np.ascontiguousarray(
        wg.transpose(1, 2, 0, 3).reshape(128, NCHUNK * 256)
    ).astype(ml_dtypes.bfloat16)
    bias_sb = np.ascontiguousarray(
        bias.astype(np.float32).reshape(2, 128).T)

    in_maps = []
    for n in range(N):
        idx_sb, wq_sb = _host_prep(offset[n], mask[n])
        in_maps.append({
            "x": np.ascontiguousarray(input[n].reshape(CIN, S).astype(np.float32)),
            "idx": idx_sb,
            "wq": wq_sb,
            "wg": wg,
            "bias": bias_sb,
        })
    res = run_bass_kernel_spmd(nc, in_maps, list(range(N_CORES)))
    out = np.stack([res.results[n]["y"] for n in range(N)])
    return out.reshape(N, COUT, HH, WW).astype(np.float32)


# revision 2
# speedup vs baseline: 1.2811x; 1.2811x over previous
"""DeformConv2d (DCNv2) on 8 Trainium2 NeuronCores.

Data-parallel over batch: one image per core.

Per-core pipeline:
  1. Build channel-last bf16 table2 in DRAM: row p = [img[:, p], img[:, p+64]]
     (channels at spatial p and at the row below). One 2KB dma_gather
     descriptor at row idx then covers all 4 bilinear neighbors
     (x-pair contiguous via elem_step=512, y-pair packed in the row).
  2. dma_gather 9*4096 sample rows -> tiles [s%128 partitions, 4*256 ch].
  3. Blend on DVE: 4 fused tensor_scalar / scalar_tensor_tensor ops with
     per-partition (= per output position) bilinear*valid*mask weights.
  4. PE transpose blended cols [s,c] -> [c,s], GEMM (contraction 2304 = 9k x
     2 c-halves x 128) accumulating in PSUM, bias add, fp32 out.

Host precomputes gather indices + folded bilinear weights from offset/mask
(small: 18*64*64 per image) and packs weight/bias into GEMM layout.
"""
import os
import sys
from contextlib import ExitStack

import numpy as np
import ml_dtypes

import concourse.bass as bass
import concourse.mybir as mybir
from concourse import bacc
from concourse.ap import AP
from concourse.tile import TileContext
from concourse.bass_utils import run_bass_kernel_spmd

N_CORES = 8
N, CIN, HH, WW = 8, 256, 64, 64
COUT = 256
KK = 9
S = HH * WW                 # 4096 output positions
TROWS = 4160                # table2 rows (>= 4097), 512 bf16 each
ROW = 512                   # elements per table2 row
ELEM = 1024                 # elements per gather descriptor (2 rows)
NQ = 4                      # s-quarters
SQ = S // NQ                # 1024 positions per quarter
CHUNK = SQ                  # idxs per dma_gather call
JL = SQ // 128              # 8 j-chunks per quarter
NIDX = KK * S               # 36864 gathers per image
NCHUNK = 18                 # contraction chunks (9 k * 2 c-halves)

bf16 = mybir.dt.bfloat16
f32 = mybir.dt.float32
i16 = mybir.dt.int16
MUL = mybir.AluOpType.mult
ADD = mybir.AluOpType.add

_PROGRAM = None


def _build_program():
    nc = bacc.Bacc("TRN2", target_bir_lowering=False, debug=False,
                   num_devices=N_CORES)

    x = nc.dram_tensor("x", [CIN, S], f32, kind="ExternalInput")
    idx_d = nc.dram_tensor("idx", [128, NIDX // 16], i16, kind="ExternalInput")
    wq_d = nc.dram_tensor("wq", [128, NQ * KK * 4 * JL], f32, kind="ExternalInput")
    wg_d = nc.dram_tensor("wg", [128, NCHUNK * 256], bf16, kind="ExternalInput")
    bias_d = nc.dram_tensor("bias", [128, 2], f32, kind="ExternalInput")
    y = nc.dram_tensor("y", [COUT, S], f32, kind="ExternalOutput")
    table2 = nc.dram_tensor("table2", [TROWS, ROW], bf16)

    # overlapping gather source AP: row i covers elements [i*512, i*512+1024)
    src_ap = AP(tensor=table2, offset=0, ap=[[ROW, TROWS - 2], [1, ELEM]])

    with TileContext(nc) as tc, ExitStack() as stk:
        const_pool = stk.enter_context(tc.tile_pool(name="const", bufs=1))
        big_pool = stk.enter_context(tc.tile_pool(name="big", bufs=3))
        imgb_pool = stk.enter_context(tc.tile_pool(name="imgb", bufs=2))
        tt_pool = stk.enter_context(tc.tile_pool(name="tt", bufs=3))
        acc_pool = stk.enter_context(tc.tile_pool(name="acc", bufs=6))
        cols_pool = stk.enter_context(tc.tile_pool(name="cols", bufs=2))
        out_pool = stk.enter_context(tc.tile_pool(name="out", bufs=3))
        ps_pool = stk.enter_context(tc.tile_pool(name="ps", bufs=3, space="PSUM"))
        pg_pool = stk.enter_context(tc.tile_pool(name="pg", bufs=4, space="PSUM"))

        # ---- constants / parameters -------------------------------------
        idx_sb = const_pool.tile([128, NIDX // 16], i16)
        nc.sync.dma_start(out=idx_sb[:], in_=idx_d[:])
        wq_sb = const_pool.tile([128, NQ * KK * 4 * JL], f32)
        nc.sync.dma_start(out=wq_sb[:], in_=wq_d[:])
        wg_sb = const_pool.tile([128, NCHUNK * 256], bf16)
        nc.sync.dma_start(out=wg_sb[:], in_=wg_d[:])
        bias_sb = const_pool.tile([128, 2], f32)
        nc.sync.dma_start(out=bias_sb[:], in_=bias_d[:])

        ident = const_pool.tile([128, 128], bf16)
        nc.vector.memset(ident[:], 1.0)
        nc.gpsimd.affine_select(
            ident[:], ident[:], pattern=[[1, 128]],
            compare_op=mybir.AluOpType.is_equal, fill=0.0,
            base=0, channel_multiplier=-1,
        )

        # ---- zero the table tail (rows 4032+: y+1 slots / pad row) ------
        ztile = const_pool.tile([128, ROW], bf16)
        nc.vector.memset(ztile[:], 0.0)
        nc.sync.dma_start(out=table2[4032:4160, :], in_=ztile[:])

        # ---- build table2: transpose image to channel-last bf16 ---------
        # table2[p, 0:256]   = img[:, p]
        # table2[p, 256:512] = img[:, p + 64]   (rows < 4032)
        for cc in range(2):
            in_sb = big_pool.tile([128, S], f32, tag="big")
            nc.sync.dma_start(out=in_sb[:], in_=x[cc * 128:(cc + 1) * 128, :])
            imgb = imgb_pool.tile([128, S], bf16)
            nc.vector.tensor_copy(imgb[:], in_sb[:])
            for g in range(4):
                tt = tt_pool.tile([128, 8, 128], bf16)
                for t in range(8):
                    ps = ps_pool.tile([128, 4, 128], bf16)
                    nc.tensor.transpose(
                        ps[:, 0, :], imgb[:, (g * 8 + t) * 128:(g * 8 + t + 1) * 128],
                        ident[:])
                    nc.scalar.copy(tt[:, t, :], ps[:, 0, :])
                hw0 = g * 1024
                # first half: rows hw0 + t*128 + p, cols cc*128..+128
                dst1 = AP(tensor=table2, offset=hw0 * ROW + cc * 128,
                          ap=[[ROW, 128], [ROW * 128, 8], [1, 128]])
                nc.sync.dma_start(out=dst1, in_=tt[:])
                # second half: rows hw0 - 64 + t*128 + p, cols 256+cc*128
                if g == 0:
                    dst2a = AP(tensor=table2, offset=256 + cc * 128,
                               ap=[[ROW, 64], [1, 128]])
                    nc.sync.dma_start(out=dst2a, in_=tt[64:128, 0, :])
                    dst2b = AP(tensor=table2, offset=64 * ROW + 256 + cc * 128,
                               ap=[[ROW, 128], [ROW * 128, 7], [1, 128]])
                    nc.sync.dma_start(out=dst2b, in_=tt[:, 1:8, :])
                else:
                    dst2 = AP(tensor=table2,
                              offset=(hw0 - 64) * ROW + 256 + cc * 128,
                              ap=[[ROW, 128], [ROW * 128, 8], [1, 128]])
                    nc.sync.dma_start(out=dst2, in_=tt[:])

        # ---- main loop ---------------------------------------------------
        for Q in range(NQ):
            cols_cs = cols_pool.tile([128, NCHUNK, SQ], bf16, tag="cols")
            for k in range(KK):
                cb = Q * KK + k
                g_t = big_pool.tile([128, JL, ELEM], bf16, tag="big")
                nc.gpsimd.dma_gather(
                    g_t[:], src_ap,
                    idx_sb[:, cb * (CHUNK // 16):(cb + 1) * (CHUNK // 16)],
                    CHUNK, CHUNK, ELEM, elem_step=ROW,
                )
                for jl in range(JL):
                    wcol = ((Q * KK + k) * 4) * JL + jl
                    # 4-neighbor blend: ACT does 2 scale-muls, DVE fuses the
                    # other 2 via scalar_tensor_tensor and adds the halves.
                    t0 = acc_pool.tile([128, 256], bf16, tag="t0")
                    nc.scalar.mul(t0[:], g_t[:, jl, 0:256],
                                  wq_sb[:, wcol:wcol + 1])
                    t2 = acc_pool.tile([128, 256], bf16, tag="t2")
                    nc.scalar.mul(t2[:], g_t[:, jl, 512:768],
                                  wq_sb[:, wcol + 2 * JL:wcol + 2 * JL + 1])
                    a = acc_pool.tile([128, 256], bf16, tag="a")
                    nc.vector.scalar_tensor_tensor(
                        a[:], g_t[:, jl, 256:512],
                        wq_sb[:, wcol + JL:wcol + JL + 1], t0[:], MUL, ADD)
                    b = acc_pool.tile([128, 256], bf16, tag="b")
                    nc.vector.scalar_tensor_tensor(
                        b[:], g_t[:, jl, 768:1024],
                        wq_sb[:, wcol + 3 * JL:wcol + 3 * JL + 1], t2[:], MUL, ADD)
                    acc = acc_pool.tile([128, 256], bf16, tag="acc")
                    nc.vector.tensor_tensor(acc[:], a[:], b[:], ADD)
                    # transpose [s,c] -> [c,s] per c-half
                    for ch in range(2):
                        ps = ps_pool.tile([128, 4, 128], bf16)
                        nc.tensor.transpose(
                            ps[:, jl % 4, :], acc[:, ch * 128:(ch + 1) * 128],
                            ident[:])
                        nc.scalar.copy(
                            cols_cs[:, k * 2 + ch, jl * 128:(jl + 1) * 128],
                            ps[:, jl % 4, :])
            for b in range(2):
                for oc in range(2):
                    po = pg_pool.tile([128, 512], f32)
                    for chunk in range(NCHUNK):
                        nc.tensor.matmul(
                            po[:],
                            wg_sb[:, chunk * 256 + oc * 128:chunk * 256 + (oc + 1) * 128],
                            cols_cs[:, chunk, b * 512:(b + 1) * 512],
                            start=(chunk == 0), stop=(chunk == NCHUNK - 1))
                    osb = out_pool.tile([128, 512], f32)
                    nc.scalar.activation(
                        osb[:], po[:], mybir.ActivationFunctionType.Identity,
                        bias=bias_sb[:, oc:oc + 1])
                    nc.sync.dma_start(
                        out=y[oc * 128:(oc + 1) * 128,
                              Q * SQ + b * 512:Q * SQ + (b + 1) * 512],
                        in_=osb[:])

    nc.compile()
    return nc


def _host_prep(offset, mask):
    """Per-image gather indices + folded bilinear weights.

    Returns idx_sb [128, NIDX//16] i16 and wq_sb [128, NQ*KK*4*JL] f32.
    """
    off = offset.reshape(KK, 2, S).astype(np.float32)
    dy = off[:, 0]
    dx = off[:, 1]
    ky = (np.arange(KK, dtype=np.float32) // 3)[:, None]
    kx = (np.arange(KK, dtype=np.float32) % 3)[:, None]
    ho = np.float32(np.arange(S) // WW)[None, :]
    wo = np.float32(np.arange(S) % WW)[None, :]
    yy = ((ky + (ho - np.float32(1.0))).astype(np.float32) + dy).astype(np.float32)
    xx = ((kx + (wo - np.float32(1.0))).astype(np.float32) + dx).astype(np.float32)
    y0f = np.floor(yy)
    x0f = np.floor(xx)
    wy1 = yy - y0f
    wx1 = xx - x0f
    wy0 = np.float32(1.0) - wy1
    wx0 = np.float32(1.0) - wx1
    y0 = y0f.astype(np.int64)
    x0 = x0f.astype(np.int64)
    vy0 = (y0 >= 0) & (y0 < HH)
    vy1 = (y0 + 1 >= 0) & (y0 + 1 < HH)
    vx0 = (x0 >= 0) & (x0 < WW)
    vx1 = (x0 + 1 >= 0) & (x0 + 1 < WW)
    m = mask.reshape(KK, S).astype(np.float32)
    W00 = wy0 * wx0 * vy0 * vx0 * m
    W10 = wy1 * wx0 * vy1 * vx0 * m
    W01 = wy0 * wx1 * vy0 * vx1 * m
    W11 = wy1 * wx1 * vy1 * vx1 * m
    sy = y0 == -1
    sx = x0 == -1
    A00 = np.where(sy, np.where(sx, W11, W10), np.where(sx, W01, W00))
    A10 = np.where(sy, 0.0, np.where(sx, W11, W10))
    A01 = np.where(sx, 0.0, np.where(sy, W11, W01))
    A11 = np.where(sy | sx, 0.0, W11)
    y0c = np.clip(y0 + sy, 0, HH - 1)
    x0c = np.clip(x0 + sx, 0, WW - 1)
    idx = (y0c * WW + x0c).astype(np.int16)          # [KK, S]
    Wq = np.stack([A00, A10, A01, A11], 1).astype(np.float32)  # [KK, 4, S]

    # pack idx: call cb = Q*KK + k covers positions i (s = Q*SQ+i);
    # i -> partition i%16, col cb*64 + i//16; replicate x8 across partitions.
    idx_q = idx.reshape(KK, NQ, SQ)                   # [k, Q, i]
    idx_q = idx_q.transpose(1, 0, 2)                  # [Q, k, i]
    cols = idx_q.reshape(NQ * KK, SQ // 16, 16)       # [cb, col, row]
    idx16 = cols.transpose(2, 0, 1).reshape(16, NQ * KK * (SQ // 16))
    idx_sb = np.tile(idx16, (8, 1))

    # pack weights: col = ((Q*KK + k)*4 + q)*JL + jl; partition p = s%128
    w = Wq.reshape(KK, 4, NQ, JL, 128)                # [k, q, Q, jl, p]
    w = w.transpose(4, 2, 0, 1, 3)                    # [p, Q, k, q, jl]
    wq_sb = np.ascontiguousarray(w.reshape(128, NQ * KK * 4 * JL))
    return idx_sb, wq_sb


def kernel(input, offset, mask, weight, bias):
    global _PROGRAM
    if _PROGRAM is None:
        _PROGRAM = _build_program()
    nc = _PROGRAM

    wg = weight.reshape(COUT, CIN, KK).astype(np.float32)
    wg = wg.transpose(1, 2, 0).reshape(2, 128, KK, COUT)
    wg = np.ascontiguousarray(
        wg.transpose(1, 2, 0, 3).reshape(128, NCHUNK * 256)
    ).astype(ml_dtypes.bfloat16)
    bias_sb = np.ascontiguousarray(
        bias.astype(np.float32).reshape(2, 128).T)

    in_maps = []
    for n in range(N):
        idx_sb, wq_sb = _host_prep(offset[n], mask[n])
        in_maps.append({
            "x": np.ascontiguousarray(input[n].reshape(CIN, S).astype(np.float32)),
            "idx": idx_sb,
            "wq": wq_sb,
            "wg": wg,
            "bias": bias_sb,
        })
    res = run_bass_kernel_spmd(nc, in_maps, list(range(N_CORES)))
    out = np.stack([res.results[n]["y"] for n in range(N)])
    return out.reshape(N, COUT, HH, WW).astype(np.float32)


# revision 5
# speedup vs baseline: 1.4308x; 1.1169x over previous
"""DeformConv2d (DCNv2) on 8 Trainium2 NeuronCores.

Data-parallel over batch: one image per core.

Per-core pipeline:
  1. Build channel-last bf16 table2 in DRAM: row p = [img[:, p], img[:, p+64]]
     (channels at spatial p and at the row below). One 2KB dma_gather
     descriptor at row idx then covers all 4 bilinear neighbors
     (x-pair contiguous via elem_step=512, y-pair packed in the row).
  2. dma_gather 9*4096 sample rows -> tiles [s%128 partitions, 4*256 ch].
  3. Blend on DVE: 4 fused tensor_scalar / scalar_tensor_tensor ops with
     per-partition (= per output position) bilinear*valid*mask weights.
  4. PE transpose blended cols [s,c] -> [c,s], GEMM (contraction 2304 = 9k x
     2 c-halves x 128) accumulating in PSUM, bias add, fp32 out.

Host precomputes gather indices + folded bilinear weights from offset/mask
(small: 18*64*64 per image) and packs weight/bias into GEMM layout.
"""
import os
import sys
from contextlib import ExitStack

import numpy as np
import ml_dtypes

import concourse.bass as bass
import concourse.mybir as mybir
from concourse import bacc
from concourse.ap import AP
from concourse.tile import TileContext
from concourse.bass_utils import run_bass_kernel_spmd

N_CORES = 8
N, CIN, HH, WW = 8, 256, 64, 64
COUT = 256
KK = 9
S = HH * WW                 # 4096 output positions
TROWS = 4160                # table2 rows (>= 4097), 512 bf16 each
ROW = 512                   # elements per table2 row
ELEM = 1024                 # elements per gather descriptor (2 rows)
NQ = 4                      # s-quarters
SQ = S // NQ                # 1024 positions per quarter
CHUNK = SQ                  # idxs per dma_gather call
JL = SQ // 128              # 8 j-chunks per quarter
NIDX = KK * S               # 36864 gathers per image
NCHUNK = 18                 # contraction chunks (9 k * 2 c-halves)

bf16 = mybir.dt.bfloat16
f32 = mybir.dt.float32
i16 = mybir.dt.int16
MUL = mybir.AluOpType.mult
ADD = mybir.AluOpType.add

_PROGRAM = None


def _build_program():
    nc = bacc.Bacc("TRN2", target_bir_lowering=False, debug=False,
                   num_devices=N_CORES)

    x = nc.dram_tensor("x", [CIN, S], f32, kind="ExternalInput")
    idx_d = nc.dram_tensor("idx", [128, NIDX // 16], i16, kind="ExternalInput")
    wq_d = nc.dram_tensor("wq", [128, NQ * KK * 4 * JL], f32, kind="ExternalInput")
    wg_d = nc.dram_tensor("wg", [128, NCHUNK * 256], bf16, kind="ExternalInput")
    bias_d = nc.dram_tensor("bias", [128, 2], f32, kind="ExternalInput")
    y = nc.dram_tensor("y", [COUT, S], f32, kind="ExternalOutput")
    table2 = nc.dram_tensor("table2", [TROWS, ROW], bf16)

    # overlapping gather source AP: row i covers elements [i*512, i*512+1024)
    src_ap = AP(tensor=table2, offset=0, ap=[[ROW, TROWS - 2], [1, ELEM]])

    with TileContext(nc) as tc, ExitStack() as stk:
        const_pool = stk.enter_context(tc.tile_pool(name="const", bufs=1))
        big_pool = stk.enter_context(tc.tile_pool(name="big", bufs=3))
        imgb_pool = stk.enter_context(tc.tile_pool(name="imgb", bufs=2))
        tt_pool = stk.enter_context(tc.tile_pool(name="tt", bufs=3))
        acc_pool = stk.enter_context(tc.tile_pool(name="acc", bufs=6))
        cols_pool = stk.enter_context(tc.tile_pool(name="cols", bufs=2))
        out_pool = stk.enter_context(tc.tile_pool(name="out", bufs=3))
        ps_pool = stk.enter_context(tc.tile_pool(name="ps", bufs=3, space="PSUM"))
        pg_pool = stk.enter_context(tc.tile_pool(name="pg", bufs=4, space="PSUM"))

        # ---- constants / parameters -------------------------------------
        idx_sb = const_pool.tile([128, NIDX // 16], i16)
        nc.sync.dma_start(out=idx_sb[:], in_=idx_d[:])
        wq_sb = const_pool.tile([128, NQ * KK * 4 * JL], f32)
        nc.sync.dma_start(out=wq_sb[:], in_=wq_d[:])
        wg_sb = const_pool.tile([128, NCHUNK * 256], bf16)
        nc.sync.dma_start(out=wg_sb[:], in_=wg_d[:])
        bias_sb = const_pool.tile([128, 2], f32)
        nc.sync.dma_start(out=bias_sb[:], in_=bias_d[:])

        ident = const_pool.tile([128, 128], bf16)
        nc.vector.memset(ident[:], 1.0)
        nc.gpsimd.affine_select(
            ident[:], ident[:], pattern=[[1, 128]],
            compare_op=mybir.AluOpType.is_equal, fill=0.0,
            base=0, channel_multiplier=-1,
        )

        # ---- zero the table tail (rows 4032+: y+1 slots / pad row) ------
        ztile = const_pool.tile([128, ROW], bf16)
        nc.vector.memset(ztile[:], 0.0)
        nc.sync.dma_start(out=table2[4032:4160, :], in_=ztile[:])

        # ---- build table2: transpose image to channel-last bf16 ---------
        # table2[p, 0:256]   = img[:, p]
        # table2[p, 256:512] = img[:, p + 64]   (rows < 4032)
        for cc in range(2):
            in_sb = big_pool.tile([128, S], f32, tag="big")
            nc.sync.dma_start(out=in_sb[:], in_=x[cc * 128:(cc + 1) * 128, :])
            imgb = imgb_pool.tile([128, S], bf16)
            nc.vector.tensor_copy(imgb[:], in_sb[:])
            for g in range(4):
                tt = tt_pool.tile([128, 8, 128], bf16)
                ps = ps_pool.tile([128, JL, 128], bf16, tag="ps")
                for t in range(8):
                    nc.tensor.transpose(
                        ps[:, t, :], imgb[:, (g * 8 + t) * 128:(g * 8 + t + 1) * 128],
                        ident[:])
                nc.scalar.copy(tt[:], ps[:])
                hw0 = g * 1024
                # first half: rows hw0 + t*128 + p, cols cc*128..+128
                dst1 = AP(tensor=table2, offset=hw0 * ROW + cc * 128,
                          ap=[[ROW, 128], [ROW * 128, 8], [1, 128]])
                nc.sync.dma_start(out=dst1, in_=tt[:])
                # second half: rows hw0 - 64 + t*128 + p, cols 256+cc*128
                if g == 0:
                    dst2a = AP(tensor=table2, offset=256 + cc * 128,
                               ap=[[ROW, 64], [1, 128]])
                    nc.sync.dma_start(out=dst2a, in_=tt[64:128, 0, :])
                    dst2b = AP(tensor=table2, offset=64 * ROW + 256 + cc * 128,
                               ap=[[ROW, 128], [ROW * 128, 7], [1, 128]])
                    nc.sync.dma_start(out=dst2b, in_=tt[:, 1:8, :])
                else:
                    dst2 = AP(tensor=table2,
                              offset=(hw0 - 64) * ROW + 256 + cc * 128,
                              ap=[[ROW, 128], [ROW * 128, 8], [1, 128]])
                    nc.sync.dma_start(out=dst2, in_=tt[:])

        # ---- main loop ---------------------------------------------------
        for Q in range(NQ):
            cols_cs = cols_pool.tile([128, NCHUNK, SQ], bf16, tag="cols")
            for k in range(KK):
                cb = Q * KK + k
                g_t = big_pool.tile([128, JL, ELEM], bf16, tag="big")
                nc.gpsimd.dma_gather(
                    g_t[:], src_ap,
                    idx_sb[:, cb * (CHUNK // 16):(cb + 1) * (CHUNK // 16)],
                    CHUNK, CHUNK, ELEM, elem_step=ROW,
                )
                ps0 = ps_pool.tile([128, JL, 128], bf16, tag="ps")
                ps1 = ps_pool.tile([128, JL, 128], bf16, tag="ps")
                for jl in range(JL):
                    wcol = ((Q * KK + k) * 4) * JL + jl
                    # separable bilinear: x-blend (both y rows at once, FD=512)
                    # then y-blend with mask folded in (FD=256).
                    # weights: q0=WX0, q1=WX1, q2=WY0*m, q3=WY1*m
                    u = acc_pool.tile([128, 512], bf16, tag="u")
                    nc.scalar.mul(u[:], g_t[:, jl, 0:512],
                                  wq_sb[:, wcol:wcol + 1])
                    nc.vector.scalar_tensor_tensor(
                        u[:], g_t[:, jl, 512:1024],
                        wq_sb[:, wcol + JL:wcol + JL + 1], u[:], MUL, ADD)
                    acc = acc_pool.tile([128, 256], bf16, tag="acc")
                    nc.scalar.mul(acc[:], u[:, 0:256],
                                  wq_sb[:, wcol + 2 * JL:wcol + 2 * JL + 1])
                    nc.vector.scalar_tensor_tensor(
                        acc[:], u[:, 256:512],
                        wq_sb[:, wcol + 3 * JL:wcol + 3 * JL + 1], acc[:],
                        MUL, ADD)
                    # transpose [s,c] -> [c,s] per c-half
                    nc.tensor.transpose(ps0[:, jl, :], acc[:, 0:128], ident[:])
                    nc.tensor.transpose(ps1[:, jl, :], acc[:, 128:256], ident[:])
                nc.scalar.copy(cols_cs[:, k * 2, :], ps0[:])
                nc.scalar.copy(cols_cs[:, k * 2 + 1, :], ps1[:])
            for b in range(2):
                for oc in range(2):
                    po = pg_pool.tile([128, 512], f32)
                    for chunk in range(NCHUNK):
                        nc.tensor.matmul(
                            po[:],
                            wg_sb[:, chunk * 256 + oc * 128:chunk * 256 + (oc + 1) * 128],
                            cols_cs[:, chunk, b * 512:(b + 1) * 512],
                            start=(chunk == 0), stop=(chunk == NCHUNK - 1))
                    osb = out_pool.tile([128, 512], f32)
                    nc.scalar.activation(
                        osb[:], po[:], mybir.ActivationFunctionType.Identity,
                        bias=bias_sb[:, oc:oc + 1])
                    nc.sync.dma_start(
                        out=y[oc * 128:(oc + 1) * 128,
                              Q * SQ + b * 512:Q * SQ + (b + 1) * 512],
                        in_=osb[:])

    nc.compile()
    return nc


def _host_prep(offset, mask):
    """Per-image gather indices + folded bilinear weights.

    Returns idx_sb [128, NIDX//16] i16 and wq_sb [128, NQ*KK*4*JL] f32.
    """
    off = offset.reshape(KK, 2, S).astype(np.float32)
    dy = off[:, 0]
    dx = off[:, 1]
    ky = (np.arange(KK, dtype=np.float32) // 3)[:, None]
    kx = (np.arange(KK, dtype=np.float32) % 3)[:, None]
    ho = np.float32(np.arange(S) // WW)[None, :]
    wo = np.float32(np.arange(S) % WW)[None, :]
    yy = ((ky + (ho - np.float32(1.0))).astype(np.float32) + dy).astype(np.float32)
    xx = ((kx + (wo - np.float32(1.0))).astype(np.float32) + dx).astype(np.float32)
    y0f = np.floor(yy)
    x0f = np.floor(xx)
    wy1 = yy - y0f
    wx1 = xx - x0f
    wy0 = np.float32(1.0) - wy1
    wx0 = np.float32(1.0) - wx1
    y0 = y0f.astype(np.int64)
    x0 = x0f.astype(np.int64)
    vy0 = (y0 >= 0) & (y0 < HH)
    vy1 = (y0 + 1 >= 0) & (y0 + 1 < HH)
    vx0 = (x0 >= 0) & (x0 < WW)
    vx1 = (x0 + 1 >= 0) & (x0 + 1 < WW)
    m = mask.reshape(KK, S).astype(np.float32)
    sy = y0 == -1
    sx = x0 == -1
    # separable weights, with the -1 edge shift folded in:
    # sample = (WY0*m)*(WX0*g(y0',x0') + WX1*g(y0',x0'+1)) + (WY1*m)*(...)
    WX0 = np.where(sx, wx1 * vx1, wx0 * vx0).astype(np.float32)
    WX1 = np.where(sx, 0.0, wx1 * vx1).astype(np.float32)
    WY0 = (np.where(sy, wy1 * vy1, wy0 * vy0) * m).astype(np.float32)
    WY1 = (np.where(sy, 0.0, wy1 * vy1) * m).astype(np.float32)
    y0c = np.clip(y0 + sy, 0, HH - 1)
    x0c = np.clip(x0 + sx, 0, WW - 1)
    idx = (y0c * WW + x0c).astype(np.int16)          # [KK, S]
    Wq = np.stack([WX0, WX1, WY0, WY1], 1).astype(np.float32)  # [KK, 4, S]

    # pack idx: call cb = Q*KK + k covers positions i (s = Q*SQ+i);
    # i -> partition i%16, col cb*64 + i//16; replicate x8 across partitions.
    idx_q = idx.reshape(KK, NQ, SQ)                   # [k, Q, i]
    idx_q = idx_q.transpose(1, 0, 2)                  # [Q, k, i]
    cols = idx_q.reshape(NQ * KK, SQ // 16, 16)       # [cb, col, row]
    idx16 = cols.transpose(2, 0, 1).reshape(16, NQ * KK * (SQ // 16))
    idx_sb = np.tile(idx16, (8, 1))

    # pack weights: col = ((Q*KK + k)*4 + q)*JL + jl; partition p = s%128
    w = Wq.reshape(KK, 4, NQ, JL, 128)                # [k, q, Q, jl, p]
    w = w.transpose(4, 2, 0, 1, 3)                    # [p, Q, k, q, jl]
    wq_sb = np.ascontiguousarray(w.reshape(128, NQ * KK * 4 * JL))
    return idx_sb, wq_sb


def kernel(input, offset, mask, weight, bias):
    global _PROGRAM
    if _PROGRAM is None:
        _PROGRAM = _build_program()
    nc = _PROGRAM

    wg = weight.reshape(COUT, CIN, KK).astype(np.float32)
    wg = wg.transpose(1, 2, 0).reshape(2, 128, KK, COUT)
    wg = np.ascontiguousarray(
        wg.transpose(1, 2, 0, 3).reshape(128, NCHUNK * 256)
    ).astype(ml_dtypes.bfloat16)
    bias_sb = np.ascontiguousarray(
        bias.astype(np.float32).reshape(2, 128).T)

    in_maps = []
    for n in range(N):
        idx_sb, wq_sb = _host_prep(offset[n], mask[n])
        in_maps.append({
            "x": np.ascontiguousarray(input[n].reshape(CIN, S).astype(np.float32)),
            "idx": idx_sb,
            "wq": wq_sb,
            "wg": wg,
            "bias": bias_sb,
        })
    res = run_bass_kernel_spmd(nc, in_maps, list(range(N_CORES)))
    out = np.stack([res.results[n]["y"] for n in range(N)])
    return out.reshape(N, COUT, HH, WW).astype(np.float32)


# revision 11
# speedup vs baseline: 1.6721x; 1.1686x over previous
"""DeformConv2d (DCNv2) on 8 Trainium2 NeuronCores.

Data-parallel over batch: one image per core.

Per-core pipeline:
  1. Build channel-last bf16 table2 in DRAM: row p = [img[:, p], img[:, p+64]]
     (channels at spatial p and at the row below). One 2KB dma_gather
     descriptor at row idx then covers all 4 bilinear neighbors
     (x-pair contiguous via elem_step=512, y-pair packed in the row).
  2. dma_gather 9*4096 sample rows -> tiles [s%128 partitions, 4*256 ch].
  3. Blend on DVE: 4 fused tensor_scalar / scalar_tensor_tensor ops with
     per-partition (= per output position) bilinear*valid*mask weights.
  4. PE transpose blended cols [s,c] -> [c,s], GEMM (contraction 2304 = 9k x
     2 c-halves x 128) accumulating in PSUM, bias add, fp32 out.

Host precomputes gather indices + folded bilinear weights from offset/mask
(small: 18*64*64 per image) and packs weight/bias into GEMM layout.
"""
import os
import sys
from contextlib import ExitStack

import numpy as np
import ml_dtypes

import concourse.bass as bass
import concourse.mybir as mybir
from concourse import bacc
from concourse.ap import AP
from concourse.tile import TileContext
from concourse.bass_utils import run_bass_kernel_spmd

N_CORES = 8
N, CIN, HH, WW = 8, 256, 64, 64
COUT = 256
KK = 9
S = HH * WW                 # 4096 output positions
TROWS = 4160                # table2 rows (>= 4097), 512 bf16 each
ROW = 512                   # elements per table2 row
ELEM = 1024                 # elements per gather descriptor (2 rows)
NQ = 4                      # s-quarters
SQ = S // NQ                # 1024 positions per quarter
CHUNK = SQ                  # idxs per dma_gather call
JL = SQ // 128              # 8 j-chunks per quarter
NIDX = KK * S               # 36864 gathers per image
NCHUNK = 18                 # contraction chunks (9 k * 2 c-halves)

bf16 = mybir.dt.bfloat16
f32 = mybir.dt.float32
i16 = mybir.dt.int16
MUL = mybir.AluOpType.mult
ADD = mybir.AluOpType.add

_PROGRAM = None


def _build_program():
    nc = bacc.Bacc("TRN2", target_bir_lowering=False, debug=False,
                   num_devices=N_CORES)

    x = nc.dram_tensor("x", [CIN, S], f32, kind="ExternalInput")
    idx_d = nc.dram_tensor("idx", [128, NIDX // 16], i16, kind="ExternalInput")
    wq_d = nc.dram_tensor("wq", [128, NQ * KK * 2 * JL], f32, kind="ExternalInput")
    dg_d = nc.dram_tensor("dg", [128, NQ * KK * JL * 2 * 128], bf16,
                          kind="ExternalInput")
    wg_d = nc.dram_tensor("wg", [128, NCHUNK * 256], bf16, kind="ExternalInput")
    bias_d = nc.dram_tensor("bias", [128, 2], f32, kind="ExternalInput")
    y = nc.dram_tensor("y", [COUT, S], f32, kind="ExternalOutput")
    table2 = nc.dram_tensor("table2", [TROWS, ROW], bf16)

    # overlapping gather source AP: row i covers elements [i*512, i*512+1024)
    src_ap = AP(tensor=table2, offset=0, ap=[[ROW, TROWS - 2], [1, ELEM]])

    with TileContext(nc) as tc, ExitStack() as stk:
        const_pool = stk.enter_context(tc.tile_pool(name="const", bufs=1))
        big_pool = stk.enter_context(tc.tile_pool(name="big", bufs=3))
        imgb_pool = stk.enter_context(tc.tile_pool(name="imgb", bufs=2))
        tt_pool = stk.enter_context(tc.tile_pool(name="tt", bufs=3))
        acc_pool = stk.enter_context(tc.tile_pool(name="acc", bufs=6))
        cols_pool = stk.enter_context(tc.tile_pool(name="cols", bufs=2))
        out_pool = stk.enter_context(tc.tile_pool(name="out", bufs=3))
        ps_pool = stk.enter_context(tc.tile_pool(name="ps", bufs=3, space="PSUM"))
        pg_pool = stk.enter_context(tc.tile_pool(name="pg", bufs=2, space="PSUM"))

        # ---- constants / parameters -------------------------------------
        idx_sb = const_pool.tile([128, NIDX // 16], i16)
        nc.sync.dma_start(out=idx_sb[:], in_=idx_d[:])
        wq_sb = const_pool.tile([128, NQ * KK * 2 * JL], f32)
        nc.sync.dma_start(out=wq_sb[:], in_=wq_d[:])
        wg_sb = const_pool.tile([128, NCHUNK * 256], bf16)
        nc.sync.dma_start(out=wg_sb[:], in_=wg_d[:])
        bias_sb = const_pool.tile([128, 2], f32)
        nc.sync.dma_start(out=bias_sb[:], in_=bias_d[:])

        ident = const_pool.tile([128, 128], bf16)
        nc.vector.memset(ident[:], 1.0)
        nc.gpsimd.affine_select(
            ident[:], ident[:], pattern=[[1, 128]],
            compare_op=mybir.AluOpType.is_equal, fill=0.0,
            base=0, channel_multiplier=-1,
        )

        # ---- zero the table tail (rows 4032+: y+1 slots / pad row) ------
        ztile = const_pool.tile([128, ROW], bf16)
        nc.vector.memset(ztile[:], 0.0)
        nc.sync.dma_start(out=table2[4032:4160, :], in_=ztile[:])

        # ---- build table2: transpose image to channel-last bf16 ---------
        # table2[p, 0:256]   = img[:, p]
        # table2[p, 256:512] = img[:, p + 64]   (rows < 4032)
        for cc in range(2):
            in_sb = big_pool.tile([128, S], f32, tag="big")
            nc.sync.dma_start(out=in_sb[:], in_=x[cc * 128:(cc + 1) * 128, :])
            imgb = imgb_pool.tile([128, S], bf16)
            nc.vector.tensor_copy(imgb[:], in_sb[:])
            for g in range(4):
                tt = tt_pool.tile([128, 8, 128], bf16)
                ps = ps_pool.tile([128, JL, 128], bf16, tag="ps")
                for t in range(8):
                    nc.tensor.transpose(
                        ps[:, t, :], imgb[:, (g * 8 + t) * 128:(g * 8 + t + 1) * 128],
                        ident[:])
                nc.scalar.copy(tt[:], ps[:])
                hw0 = g * 1024
                # first half: rows hw0 + t*128 + p, cols cc*128..+128
                dst1 = AP(tensor=table2, offset=hw0 * ROW + cc * 128,
                          ap=[[ROW, 128], [ROW * 128, 8], [1, 128]])
                nc.sync.dma_start(out=dst1, in_=tt[:])
                # second half: rows hw0 - 64 + t*128 + p, cols 256+cc*128
                if g == 0:
                    dst2a = AP(tensor=table2, offset=256 + cc * 128,
                               ap=[[ROW, 64], [1, 128]])
                    nc.sync.dma_start(out=dst2a, in_=tt[64:128, 0, :])
                    dst2b = AP(tensor=table2, offset=64 * ROW + 256 + cc * 128,
                               ap=[[ROW, 128], [ROW * 128, 7], [1, 128]])
                    nc.sync.dma_start(out=dst2b, in_=tt[:, 1:8, :])
                else:
                    dst2 = AP(tensor=table2,
                              offset=(hw0 - 64) * ROW + 256 + cc * 128,
                              ap=[[ROW, 128], [ROW * 128, 8], [1, 128]])
                    nc.sync.dma_start(out=dst2, in_=tt[:])

        # ---- main loop ---------------------------------------------------
        for Q in range(NQ):
            cols_cs = cols_pool.tile([128, NCHUNK, SQ], bf16, tag="cols")
            for k in range(KK):
                cb = Q * KK + k
                g_t = big_pool.tile([128, JL, ELEM], bf16, tag="big")
                nc.gpsimd.dma_gather(
                    g_t[:], src_ap,
                    idx_sb[:, cb * (CHUNK // 16):(cb + 1) * (CHUNK // 16)],
                    CHUNK, CHUNK, ELEM, elem_step=ROW,
                )
                dg_t = tt_pool.tile([128, JL, 2, 128], bf16, tag="dg")
                nc.sync.dma_start(
                    out=dg_t[:],
                    in_=dg_d[:, cb * JL * 256:(cb + 1) * JL * 256])
                ps0 = ps_pool.tile([128, JL, 128], f32, tag="ps")
                ps1 = ps_pool.tile([128, JL, 128], f32, tag="ps")
                for jl in range(JL):
                    wcol = ((Q * KK + k) * 2) * JL + jl
                    # separable bilinear: x-blend (both y rows at once, FD=512)
                    # on ACT+DVE with per-partition scalars, then the y-blend
                    # (mask folded) fused into the PE transpose as matmuls
                    # against host-built diag(WY*m) tiles, PSUM-accumulated.
                    u = acc_pool.tile([128, 512], bf16, tag="u")
                    nc.scalar.mul(u[:], g_t[:, jl, 0:512],
                                  wq_sb[:, wcol:wcol + 1])
                    nc.vector.scalar_tensor_tensor(
                        u[:], g_t[:, jl, 512:1024],
                        wq_sb[:, wcol + JL:wcol + JL + 1], u[:], MUL, ADD)
                    # u = [u_y0 (256ch), u_y1 (256ch)]
                    for ch, psX in ((0, ps0), (1, ps1)):
                        nc.tensor.matmul(
                            psX[:, jl, :], u[:, ch * 128:ch * 128 + 128],
                            dg_t[:, jl, 0, :], start=True, stop=False)
                        nc.tensor.matmul(
                            psX[:, jl, :], u[:, 256 + ch * 128:256 + ch * 128 + 128],
                            dg_t[:, jl, 1, :], start=False, stop=True)
                nc.vector.tensor_copy(cols_cs[:, k * 2, :], ps0[:])
                nc.vector.tensor_copy(cols_cs[:, k * 2 + 1, :], ps1[:])
            for b in range(2):
                for oc in range(2):
                    po = pg_pool.tile([128, 512], f32)
                    for chunk in range(NCHUNK):
                        nc.tensor.matmul(
                            po[:],
                            wg_sb[:, chunk * 256 + oc * 128:chunk * 256 + (oc + 1) * 128],
                            cols_cs[:, chunk, b * 512:(b + 1) * 512],
                            start=(chunk == 0), stop=(chunk == NCHUNK - 1))
                    osb = out_pool.tile([128, 512], f32)
                    nc.scalar.activation(
                        osb[:], po[:], mybir.ActivationFunctionType.Identity,
                        bias=bias_sb[:, oc:oc + 1])
                    nc.sync.dma_start(
                        out=y[oc * 128:(oc + 1) * 128,
                              Q * SQ + b * 512:Q * SQ + (b + 1) * 512],
                        in_=osb[:])

    nc.compile()
    return nc


def _host_prep(offset, mask):
    """Per-image gather indices + folded bilinear weights.

    Returns idx_sb [128, NIDX//16] i16 and wq_sb [128, NQ*KK*4*JL] f32.
    """
    off = offset.reshape(KK, 2, S).astype(np.float32)
    dy = off[:, 0]
    dx = off[:, 1]
    ky = (np.arange(KK, dtype=np.float32) // 3)[:, None]
    kx = (np.arange(KK, dtype=np.float32) % 3)[:, None]
    ho = np.float32(np.arange(S) // WW)[None, :]
    wo = np.float32(np.arange(S) % WW)[None, :]
    yy = ((ky + (ho - np.float32(1.0))).astype(np.float32) + dy).astype(np.float32)
    xx = ((kx + (wo - np.float32(1.0))).astype(np.float32) + dx).astype(np.float32)
    y0f = np.floor(yy)
    x0f = np.floor(xx)
    wy1 = yy - y0f
    wx1 = xx - x0f
    wy0 = np.float32(1.0) - wy1
    wx0 = np.float32(1.0) - wx1
    y0 = y0f.astype(np.int64)
    x0 = x0f.astype(np.int64)
    vy0 = (y0 >= 0) & (y0 < HH)
    vy1 = (y0 + 1 >= 0) & (y0 + 1 < HH)
    vx0 = (x0 >= 0) & (x0 < WW)
    vx1 = (x0 + 1 >= 0) & (x0 + 1 < WW)
    m = mask.reshape(KK, S).astype(np.float32)
    sy = y0 == -1
    sx = x0 == -1
    # separable weights, with the -1 edge shift folded in:
    # sample = (WY0*m)*(WX0*g(y0',x0') + WX1*g(y0',x0'+1)) + (WY1*m)*(...)
    WX0 = np.where(sx, wx1 * vx1, wx0 * vx0).astype(np.float32)
    WX1 = np.where(sx, 0.0, wx1 * vx1).astype(np.float32)
    WY0 = (np.where(sy, wy1 * vy1, wy0 * vy0) * m).astype(np.float32)
    WY1 = (np.where(sy, 0.0, wy1 * vy1) * m).astype(np.float32)
    y0c = np.clip(y0 + sy, 0, HH - 1)
    x0c = np.clip(x0 + sx, 0, WW - 1)
    idx = (y0c * WW + x0c).astype(np.int16)          # [KK, S]

    # pack idx: call cb = Q*KK + k covers positions i (s = Q*SQ+i);
    # i -> partition i%16, col cb*64 + i//16; replicate x8 across partitions.
    idx_q = idx.reshape(KK, NQ, SQ)                   # [k, Q, i]
    idx_q = idx_q.transpose(1, 0, 2)                  # [Q, k, i]
    cols = idx_q.reshape(NQ * KK, SQ // 16, 16)       # [cb, col, row]
    idx16 = cols.transpose(2, 0, 1).reshape(16, NQ * KK * (SQ // 16))
    idx_sb = np.tile(idx16, (8, 1))

    # pack x-weights: col = ((Q*KK + k)*2 + q)*JL + jl; partition p = s%128
    w = np.stack([WX0, WX1], 1).reshape(KK, 2, NQ, JL, 128)
    w = w.transpose(4, 2, 0, 1, 3)                    # [p, Q, k, q, jl]
    wq_sb = np.ascontiguousarray(w.reshape(128, NQ * KK * 2 * JL))

    # diag(WY*m) tiles for the PE y-blend: [cb, jl, y, p, f] with the
    # weight for s = Q*SQ + jl*128 + p on the diagonal (p == f).
    wy = np.stack([WY0, WY1], 1).reshape(KK, 2, NQ, JL, 128)
    wy = wy.transpose(2, 0, 3, 1, 4)                  # [Q, k, jl, y, p]
    A = np.zeros((NQ * KK, JL, 2, 128, 128), np.float32)
    ar = np.arange(128)
    A[..., ar, ar] = wy.reshape(NQ * KK, JL, 2, 128)
    dg_sb = np.ascontiguousarray(
        A.transpose(3, 0, 1, 2, 4).reshape(128, NQ * KK * JL * 2 * 128)
    ).astype(ml_dtypes.bfloat16)
    return idx_sb, wq_sb, dg_sb


def build_in_maps(input, offset, mask, weight, bias):
    wg = weight.reshape(COUT, CIN, KK).astype(np.float32)
    wg = wg.transpose(1, 2, 0).reshape(2, 128, KK, COUT)
    wg = np.ascontiguousarray(
        wg.transpose(1, 2, 0, 3).reshape(128, NCHUNK * 256)
    ).astype(ml_dtypes.bfloat16)
    bias_sb = np.ascontiguousarray(
        bias.astype(np.float32).reshape(2, 128).T)

    in_maps = []
    for n in range(N):
        idx_sb, wq_sb, dg_sb = _host_prep(offset[n], mask[n])
        in_maps.append({
            "x": np.ascontiguousarray(input[n].reshape(CIN, S).astype(np.float32)),
            "idx": idx_sb,
            "wq": wq_sb,
            "dg": dg_sb,
            "wg": wg,
            "bias": bias_sb,
        })
    return in_maps


def kernel(input, offset, mask, weight, bias):
    global _PROGRAM
    if _PROGRAM is None:
        _PROGRAM = _build_program()
    nc = _PROGRAM
    in_maps = build_in_maps(input, offset, mask, weight, bias)
    res = run_bass_kernel_spmd(nc, in_maps, list(range(N_CORES)))
    out = np.stack([res.results[n]["y"] for n in range(N)])
    return out.reshape(N, COUT, HH, WW).astype(np.float32)


# revision 15
# speedup vs baseline: 1.7764x; 1.0624x over previous
"""DeformConv2d (DCNv2) on 8 Trainium2 NeuronCores.

Data-parallel over batch: one image per core.

Per-core pipeline:
  1. Build channel-last bf16 table2 in DRAM: row p = [img[:, p], img[:, p+64]]
     (channels at spatial p and at the row below). One 2KB dma_gather
     descriptor at row idx then covers all 4 bilinear neighbors
     (x-pair contiguous via elem_step=512, y-pair packed in the row).
  2. dma_gather 9*4096 sample rows -> tiles [s%128 partitions, 4*256 ch].
  3. Blend on DVE: 4 fused tensor_scalar / scalar_tensor_tensor ops with
     per-partition (= per output position) bilinear*valid*mask weights.
  4. PE transpose blended cols [s,c] -> [c,s], GEMM (contraction 2304 = 9k x
     2 c-halves x 128) accumulating in PSUM, bias add, fp32 out.

Host precomputes gather indices + folded bilinear weights from offset/mask
(small: 18*64*64 per image) and packs weight/bias into GEMM layout.
"""
import os
import sys
from contextlib import ExitStack

import numpy as np
import ml_dtypes

import concourse.bass as bass
import concourse.mybir as mybir
from concourse import bacc
from concourse.ap import AP
from concourse.tile import TileContext
from concourse.bass_utils import run_bass_kernel_spmd

N_CORES = 8
N, CIN, HH, WW = 8, 256, 64, 64
COUT = 256
KK = 9
S = HH * WW                 # 4096 output positions
TROWS = 4160                # table2 rows (>= 4097), 512 bf16 each
ROW = 512                   # elements per table2 row
ELEM = 1024                 # elements per gather descriptor (2 rows)
NQ = 4                      # s-quarters
SQ = S // NQ                # 1024 positions per quarter
CHUNK = SQ                  # idxs per dma_gather call
JL = SQ // 128              # 8 j-chunks per quarter
NIDX = KK * S               # 36864 gathers per image
NCHUNK = 18                 # contraction chunks (9 k * 2 c-halves)

bf16 = mybir.dt.bfloat16
f32 = mybir.dt.float32
i16 = mybir.dt.int16
MUL = mybir.AluOpType.mult
ADD = mybir.AluOpType.add

_PROGRAM = None


def _build_program():
    nc = bacc.Bacc("TRN2", target_bir_lowering=False, debug=False,
                   num_devices=N_CORES)

    x = nc.dram_tensor("x", [CIN, S], f32, kind="ExternalInput")
    idx_d = nc.dram_tensor("idx", [128, NIDX // 16], i16, kind="ExternalInput")
    wq_d = nc.dram_tensor("wq", [128, NQ * KK * 2 * JL], f32, kind="ExternalInput")
    dg_d = nc.dram_tensor("dg", [128, NQ * KK * JL * 2 * 128], bf16,
                          kind="ExternalInput")
    wg_d = nc.dram_tensor("wg", [128, NCHUNK * 256], bf16, kind="ExternalInput")
    bias_d = nc.dram_tensor("bias", [128, 2], f32, kind="ExternalInput")
    y = nc.dram_tensor("y", [COUT, S], f32, kind="ExternalOutput")
    table2 = nc.dram_tensor("table2", [TROWS, ROW], bf16)

    # overlapping gather source AP: row i covers elements [i*512, i*512+1024)
    src_ap = AP(tensor=table2, offset=0, ap=[[ROW, TROWS - 2], [1, ELEM]])

    with TileContext(nc) as tc, ExitStack() as stk:
        const_pool = stk.enter_context(tc.tile_pool(name="const", bufs=1))
        big_pool = stk.enter_context(tc.tile_pool(name="big", bufs=4))
        imgb_pool = stk.enter_context(tc.tile_pool(name="imgb", bufs=1))
        tt_pool = stk.enter_context(tc.tile_pool(name="tt", bufs=3))
        acc_pool = stk.enter_context(tc.tile_pool(name="acc", bufs=6))
        cols_pool = stk.enter_context(tc.tile_pool(name="cols", bufs=2))
        out_pool = stk.enter_context(tc.tile_pool(name="out", bufs=3))
        ps_pool = stk.enter_context(tc.tile_pool(name="ps", bufs=2, space="PSUM"))
        pg_pool = stk.enter_context(tc.tile_pool(name="pg", bufs=1, space="PSUM"))

        # ---- constants / parameters -------------------------------------
        idx_sb = const_pool.tile([128, NIDX // 16], i16)
        nc.sync.dma_start(out=idx_sb[:], in_=idx_d[:])
        wq_sb = const_pool.tile([128, NQ * KK * 2 * JL], f32)
        nc.sync.dma_start(out=wq_sb[:], in_=wq_d[:])
        wg_sb = const_pool.tile([128, NCHUNK * 256], bf16)
        nc.sync.dma_start(out=wg_sb[:], in_=wg_d[:])
        bias_sb = const_pool.tile([128, 2], f32)
        nc.sync.dma_start(out=bias_sb[:], in_=bias_d[:])

        ident = const_pool.tile([128, 128], bf16)
        nc.vector.memset(ident[:], 1.0)
        nc.gpsimd.affine_select(
            ident[:], ident[:], pattern=[[1, 128]],
            compare_op=mybir.AluOpType.is_equal, fill=0.0,
            base=0, channel_multiplier=-1,
        )

        # ---- zero the table tail (rows 4032+: y+1 slots / pad row) ------
        ztile = const_pool.tile([128, ROW], bf16)
        nc.vector.memset(ztile[:], 0.0)
        nc.sync.dma_start(out=table2[4032:4160, :], in_=ztile[:])

        # ---- build table2: transpose image to channel-last bf16 ---------
        # table2[p, 0:256]   = img[:, p]
        # table2[p, 256:512] = img[:, p + 64]   (rows < 4032)
        for cc in range(2):
            imgb = imgb_pool.tile([128, S], bf16)
            nc.gpsimd.dma_start(out=imgb[:], in_=x[cc * 128:(cc + 1) * 128, :])
            for g in range(4):
                tt = tt_pool.tile([128, 8, 128], bf16)
                ps = ps_pool.tile([128, JL, 128], bf16, tag="ps")
                for t in range(8):
                    nc.tensor.transpose(
                        ps[:, t, :], imgb[:, (g * 8 + t) * 128:(g * 8 + t + 1) * 128],
                        ident[:])
                nc.scalar.copy(tt[:], ps[:])
                hw0 = g * 1024
                # first half: rows hw0 + t*128 + p, cols cc*128..+128
                dst1 = AP(tensor=table2, offset=hw0 * ROW + cc * 128,
                          ap=[[ROW, 128], [ROW * 128, 8], [1, 128]])
                nc.sync.dma_start(out=dst1, in_=tt[:])
                # second half: rows hw0 - 64 + t*128 + p, cols 256+cc*128
                if g == 0:
                    dst2a = AP(tensor=table2, offset=256 + cc * 128,
                               ap=[[ROW, 64], [1, 128]])
                    nc.sync.dma_start(out=dst2a, in_=tt[64:128, 0, :])
                    dst2b = AP(tensor=table2, offset=64 * ROW + 256 + cc * 128,
                               ap=[[ROW, 128], [ROW * 128, 7], [1, 128]])
                    nc.sync.dma_start(out=dst2b, in_=tt[:, 1:8, :])
                else:
                    dst2 = AP(tensor=table2,
                              offset=(hw0 - 64) * ROW + 256 + cc * 128,
                              ap=[[ROW, 128], [ROW * 128, 8], [1, 128]])
                    nc.sync.dma_start(out=dst2, in_=tt[:])

        # ---- main loop ---------------------------------------------------
        for Q in range(NQ):
            cols_cs = cols_pool.tile([128, NCHUNK, SQ], bf16, tag="cols")
            pos = [pg_pool.tile([128, 512], f32, name=f"po{i}_{Q}", tag=f"po{i}")
                   for i in range(4)]
            for k in range(KK):
                cb = Q * KK + k
                g_t = big_pool.tile([128, JL, ELEM], bf16, tag="big")
                nc.gpsimd.dma_gather(
                    g_t[:], src_ap,
                    idx_sb[:, cb * (CHUNK // 16):(cb + 1) * (CHUNK // 16)],
                    CHUNK, CHUNK, ELEM, elem_step=ROW,
                )
                dg_t = tt_pool.tile([128, JL, 2, 128], bf16, tag="dg")
                nc.sync.dma_start(
                    out=dg_t[:],
                    in_=dg_d[:, cb * JL * 256:(cb + 1) * JL * 256])
                ps0 = ps_pool.tile([128, JL, 128], f32, tag="ps")
                ps1 = ps_pool.tile([128, JL, 128], f32, tag="ps")
                for jl in range(JL):
                    wcol = ((Q * KK + k) * 2) * JL + jl
                    # separable bilinear: x-blend (both y rows at once, FD=512)
                    # on ACT+DVE with per-partition scalars, then the y-blend
                    # (mask folded) fused into the PE transpose as matmuls
                    # against host-built diag(WY*m) tiles, PSUM-accumulated.
                    u = acc_pool.tile([128, 512], bf16, tag="u")
                    nc.scalar.mul(u[:], g_t[:, jl, 0:512],
                                  wq_sb[:, wcol:wcol + 1])
                    nc.vector.scalar_tensor_tensor(
                        u[:], g_t[:, jl, 512:1024],
                        wq_sb[:, wcol + JL:wcol + JL + 1], u[:], MUL, ADD)
                    # u = [u_y0 (256ch), u_y1 (256ch)]
                    for ch, psX in ((0, ps0), (1, ps1)):
                        nc.tensor.matmul(
                            psX[:, jl, :], u[:, ch * 128:ch * 128 + 128],
                            dg_t[:, jl, 0, :], start=True, stop=False)
                        nc.tensor.matmul(
                            psX[:, jl, :], u[:, 256 + ch * 128:256 + ch * 128 + 128],
                            dg_t[:, jl, 1, :], start=False, stop=True)
                nc.vector.tensor_copy(cols_cs[:, k * 2, :], ps0[:])
                nc.vector.tensor_copy(cols_cs[:, k * 2 + 1, :], ps1[:])
                for ch in range(2):
                    chunk = k * 2 + ch
                    for b in range(2):
                        for oc in range(2):
                            nc.tensor.matmul(
                                pos[b * 2 + oc][:],
                                wg_sb[:, chunk * 256 + oc * 128:chunk * 256 + (oc + 1) * 128],
                                cols_cs[:, chunk, b * 512:(b + 1) * 512],
                                start=(chunk == 0), stop=(chunk == NCHUNK - 1))
            for b in range(2):
                for oc in range(2):
                    osb = out_pool.tile([128, 512], f32)
                    nc.scalar.activation(
                        osb[:], pos[b * 2 + oc][:],
                        mybir.ActivationFunctionType.Identity,
                        bias=bias_sb[:, oc:oc + 1])
                    nc.sync.dma_start(
                        out=y[oc * 128:(oc + 1) * 128,
                              Q * SQ + b * 512:Q * SQ + (b + 1) * 512],
                        in_=osb[:])

    nc.compile()
    return nc


def _host_prep(offset, mask):
    """Per-image gather indices + folded bilinear weights.

    Returns idx_sb [128, NIDX//16] i16 and wq_sb [128, NQ*KK*4*JL] f32.
    """
    off = offset.reshape(KK, 2, S).astype(np.float32)
    dy = off[:, 0]
    dx = off[:, 1]
    ky = (np.arange(KK, dtype=np.float32) // 3)[:, None]
    kx = (np.arange(KK, dtype=np.float32) % 3)[:, None]
    ho = np.float32(np.arange(S) // WW)[None, :]
    wo = np.float32(np.arange(S) % WW)[None, :]
    yy = ((ky + (ho - np.float32(1.0))).astype(np.float32) + dy).astype(np.float32)
    xx = ((kx + (wo - np.float32(1.0))).astype(np.float32) + dx).astype(np.float32)
    y0f = np.floor(yy)
    x0f = np.floor(xx)
    wy1 = yy - y0f
    wx1 = xx - x0f
    wy0 = np.float32(1.0) - wy1
    wx0 = np.float32(1.0) - wx1
    y0 = y0f.astype(np.int64)
    x0 = x0f.astype(np.int64)
    vy0 = (y0 >= 0) & (y0 < HH)
    vy1 = (y0 + 1 >= 0) & (y0 + 1 < HH)
    vx0 = (x0 >= 0) & (x0 < WW)
    vx1 = (x0 + 1 >= 0) & (x0 + 1 < WW)
    m = mask.reshape(KK, S).astype(np.float32)
    sy = y0 == -1
    sx = x0 == -1
    # separable weights, with the -1 edge shift folded in:
    # sample = (WY0*m)*(WX0*g(y0',x0') + WX1*g(y0',x0'+1)) + (WY1*m)*(...)
    WX0 = np.where(sx, wx1 * vx1, wx0 * vx0).astype(np.float32)
    WX1 = np.where(sx, 0.0, wx1 * vx1).astype(np.float32)
    WY0 = (np.where(sy, wy1 * vy1, wy0 * vy0) * m).astype(np.float32)
    WY1 = (np.where(sy, 0.0, wy1 * vy1) * m).astype(np.float32)
    y0c = np.clip(y0 + sy, 0, HH - 1)
    x0c = np.clip(x0 + sx, 0, WW - 1)
    idx = (y0c * WW + x0c).astype(np.int16)          # [KK, S]

    # pack idx: call cb = Q*KK + k covers positions i (s = Q*SQ+i);
    # i -> partition i%16, col cb*64 + i//16; replicate x8 across partitions.
    idx_q = idx.reshape(KK, NQ, SQ)                   # [k, Q, i]
    idx_q = idx_q.transpose(1, 0, 2)                  # [Q, k, i]
    cols = idx_q.reshape(NQ * KK, SQ // 16, 16)       # [cb, col, row]
    idx16 = cols.transpose(2, 0, 1).reshape(16, NQ * KK * (SQ // 16))
    idx_sb = np.tile(idx16, (8, 1))

    # pack x-weights: col = ((Q*KK + k)*2 + q)*JL + jl; partition p = s%128
    w = np.stack([WX0, WX1], 1).reshape(KK, 2, NQ, JL, 128)
    w = w.transpose(4, 2, 0, 1, 3)                    # [p, Q, k, q, jl]
    wq_sb = np.ascontiguousarray(w.reshape(128, NQ * KK * 2 * JL))

    # diag(WY*m) tiles for the PE y-blend: [cb, jl, y, p, f] with the
    # weight for s = Q*SQ + jl*128 + p on the diagonal (p == f).
    wy = np.stack([WY0, WY1], 1).reshape(KK, 2, NQ, JL, 128)
    wy = wy.transpose(2, 0, 3, 1, 4)                  # [Q, k, jl, y, p]
    A = np.zeros((NQ * KK, JL, 2, 128, 128), np.float32)
    ar = np.arange(128)
    A[..., ar, ar] = wy.reshape(NQ * KK, JL, 2, 128)
    dg_sb = np.ascontiguousarray(
        A.transpose(3, 0, 1, 2, 4).reshape(128, NQ * KK * JL * 2 * 128)
    ).astype(ml_dtypes.bfloat16)
    return idx_sb, wq_sb, dg_sb


def build_in_maps(input, offset, mask, weight, bias):
    wg = weight.reshape(COUT, CIN, KK).astype(np.float32)
    wg = wg.transpose(1, 2, 0).reshape(2, 128, KK, COUT)
    wg = np.ascontiguousarray(
        wg.transpose(1, 2, 0, 3).reshape(128, NCHUNK * 256)
    ).astype(ml_dtypes.bfloat16)
    bias_sb = np.ascontiguousarray(
        bias.astype(np.float32).reshape(2, 128).T)

    in_maps = []
    for n in range(N):
        idx_sb, wq_sb, dg_sb = _host_prep(offset[n], mask[n])
        in_maps.append({
            "x": np.ascontiguousarray(input[n].reshape(CIN, S).astype(np.float32)),
            "idx": idx_sb,
            "wq": wq_sb,
            "dg": dg_sb,
            "wg": wg,
            "bias": bias_sb,
        })
    return in_maps


def kernel(input, offset, mask, weight, bias):
    global _PROGRAM
    if _PROGRAM is None:
        _PROGRAM = _build_program()
    nc = _PROGRAM
    in_maps = build_in_maps(input, offset, mask, weight, bias)
    res = run_bass_kernel_spmd(nc, in_maps, list(range(N_CORES)))
    out = np.stack([res.results[n]["y"] for n in range(N)])
    return out.reshape(N, COUT, HH, WW).astype(np.float32)
